# revision 13
# baseline (speedup 1.0000x reference)
"""Trainium2 Bass kernel for nn_AttentionDecoder (bf16 rewrite).

Key insight (from the reference): the per-step attention score adds a
per-batch scalar (sa) to every element of the row before softmax;
softmax is shift-invariant, so the attention weights -- and the context
vector ci -- are identical for all 64 decode steps.  The computation
collapses to:

  Phase A (streams h once):
     twh[b,t] = sum_h tanh(h[b,t,:] @ W_h_a.T)[h] * wa1[h]
     e        = exp(twh)            (unnormalized; |twh| <= ~10)
     ci[b,:]  = (e @ h[b]) / sum(e)
     s0       = tanh(h[:,0,:] @ W_init.T + b_init)
  Phase B (64 sequential GRU+RNN steps, batch=8 per core):
     si = GRU(cat(ci,y), s);  yi = softmax(tanh(RNN(cat(ci,si), y)))

Perf notes vs the fp32 baseline (1.14 ms):
  * All matmuls in bf16: fp32 matmuls run twice (fp32_mode=LOW/HIGH, two
    LDWEIGHTS+MATMUL pairs each) and disable FWL fast weight load.  bf16
    is single-pass with FWL -- production-measured ~81 ns/MM at N=128.
  * Activation table set 0 ("exp_and_others") holds BOTH exp and tanh:
    native Exp for both softmaxes (no tanh-identity emulation dance) with
    accum_out giving row sums for free.  Sigmoid is emulated as
    0.5*tanh(0.5x)+0.5 (one fused tensor_scalar) so set 2 is never
    touched -- exactly one ACT_TABLE_LOAD in the whole kernel.
  * Zero GpSimd use (baseline lost ~190us to gpsimd semaphore overhead).
    Partition reductions/broadcasts use k=1 matmuls against ones.
  * Phase B software-pipelined: step t's s-dependent gate matmuls are
    emitted before step t-1's softmax tail so the PE stays busy.

Sharding: data-parallel over batch, 8 batches per core, weights
replicated; h is cast to bf16 host-side (tolerance is 2e-2).
"""

import numpy as np
import ml_dtypes

B, T, D, H, DO, L = 64, 2048, 256, 256, 128, 64
NC = 8           # cores
BL = B // NC     # batches per core = 8
NT = T // 128    # 16 t-chunks

_CACHE = {}


def _build_program():
    import concourse.bass as bass
    import concourse.bacc as bacc
    import concourse.mybir as mybir
    import concourse.tile as tile

    dt = mybir.dt
    F32 = dt.float32
    BF16 = dt.bfloat16
    AF = mybir.ActivationFunctionType
    OP = mybir.AluOpType
    AX = mybir.AxisListType

    nc = bacc.Bacc("TRN2", target_bir_lowering=False, debug=False, num_devices=NC)

    # ---- DRAM I/O ------------------------------------------------------
    h_d = nc.dram_tensor("h", (BL, T, D), BF16, kind="ExternalInput").ap()
    whaT_d = nc.dram_tensor("whaT", (128, 512), BF16, kind="ExternalInput").ap()
    wa1r_d = nc.dram_tensor("wa1r", (128, 256), BF16, kind="ExternalInput").ap()
    winitT_d = nc.dram_tensor("winitT", (128, 512), BF16, kind="ExternalInput").ap()
    binit_d = nc.dram_tensor("binit", (128, 2), F32, kind="ExternalInput").ap()
    wgsT_d = nc.dram_tensor("wgsT", (128, 1536), BF16, kind="ExternalInput").ap()
    wgyT_d = nc.dram_tensor("wgyT", (128, 768), BF16, kind="ExternalInput").ap()
    wgciT_d = nc.dram_tensor("wgciT", (128, 1536), BF16, kind="ExternalInput").ap()
    wrsT_d = nc.dram_tensor("wrsT", (128, 256), BF16, kind="ExternalInput").ap()
    wryT_d = nc.dram_tensor("wryT", (128, 128), BF16, kind="ExternalInput").ap()
    wrciT_d = nc.dram_tensor("wrciT", (128, 256), BF16, kind="ExternalInput").ap()
    biasgT_d = nc.dram_tensor("biasgT", (128, 6), F32, kind="ExternalInput").ap()
    biasrT_d = nc.dram_tensor("biasrT", (128, 1), F32, kind="ExternalInput").ap()
    ident_d = nc.dram_tensor("ident", (128, 128), BF16, kind="ExternalInput").ap()
    onescol_d = nc.dram_tensor("onescol", (128, 1), BF16, kind="ExternalInput").ap()
    onesrow_d = nc.dram_tensor("onesrow", (1, 128), BF16, kind="ExternalInput").ap()
    out_d = nc.dram_tensor("out", (128, BL * L), BF16, kind="ExternalOutput").ap()

    # ---- persistent SBUF ----------------------------------------------
    whaT = nc.alloc_sbuf_tensor("whaT_sb", [128, 512], BF16).ap()
    wa1r = nc.alloc_sbuf_tensor("wa1r_sb", [128, 256], BF16).ap()
    winitT = nc.alloc_sbuf_tensor("winitT_sb", [128, 512], BF16).ap()
    binit = nc.alloc_sbuf_tensor("binit_sb", [128, 2], F32).ap()
    wgsT = nc.alloc_sbuf_tensor("wgsT_sb", [128, 1536], BF16).ap()
    wgyT = nc.alloc_sbuf_tensor("wgyT_sb", [128, 768], BF16).ap()
    wgciT = nc.alloc_sbuf_tensor("wgciT_sb", [128, 1536], BF16).ap()
    wrsT = nc.alloc_sbuf_tensor("wrsT_sb", [128, 256], BF16).ap()
    wryT = nc.alloc_sbuf_tensor("wryT_sb", [128, 128], BF16).ap()
    wrciT = nc.alloc_sbuf_tensor("wrciT_sb", [128, 256], BF16).ap()
    biasgT = nc.alloc_sbuf_tensor("biasgT_sb", [128, 6], F32).ap()
    biasrT = nc.alloc_sbuf_tensor("biasrT_sb", [128, 1], F32).ap()
    ident = nc.alloc_sbuf_tensor("ident_sb", [128, 128], BF16).ap()
    onescol = nc.alloc_sbuf_tensor("onescol_sb", [128, 1], BF16).ap()
    onesrow = nc.alloc_sbuf_tensor("onesrow_sb", [1, 128], BF16).ap()

    h0T = nc.alloc_sbuf_tensor("h0T", [128, 16], BF16).ap()      # h[:,0,:] cols c*8+b
    partials = nc.alloc_sbuf_tensor("partials", [128, 8], F32).ap()
    ciT = nc.alloc_sbuf_tensor("ciT", [128, 16], BF16).ap()      # cols c*8+b
    s0T = nc.alloc_sbuf_tensor("s0T", [128, 16], BF16).ap()
    constgT = nc.alloc_sbuf_tensor("constgT", [128, 48], F32).ap()
    constrT = nc.alloc_sbuf_tensor("constrT", [128, 8], F32).ap()
    out_all = nc.alloc_sbuf_tensor("out_all", [128, BL * L], BF16).ap()

    with tile.TileContext(nc) as tc:
        # weight loads
        for sb, dr in [(whaT, whaT_d), (wa1r, wa1r_d), (winitT, winitT_d),
                       (binit, binit_d), (wgsT, wgsT_d), (wgyT, wgyT_d),
                       (wgciT, wgciT_d), (wrsT, wrsT_d), (wryT, wryT_d),
                       (wrciT, wrciT_d), (biasgT, biasgT_d), (biasrT, biasrT_d),
                       (ident, ident_d), (onescol, onescol_d),
                       (onesrow, onesrow_d)]:
            nc.sync.dma_start(sb[:, :], dr[:, :])

        # ================= Phase A =================
        with tc.tile_pool(name="pcit", bufs=1, space="PSUM") as pcit_pool:
          pciT0 = pcit_pool.tile([128, 8], F32, name="pciT0", tag="pciT0")
          pciT1 = pcit_pool.tile([128, 8], F32, name="pciT1", tag="pciT1")
          with tc.tile_pool(name="hnat", bufs=24) as hnat_pool, \
             tc.tile_pool(name="hts", bufs=6) as ht_pool, \
             tc.tile_pool(name="sba", bufs=3) as sba_pool, \
             tc.tile_pool(name="smalla", bufs=3) as sm_pool, \
             tc.tile_pool(name="ptr", bufs=2, space="PSUM") as ptr_pool, \
             tc.tile_pool(name="pwh", bufs=2, space="PSUM") as pwh_pool, \
             tc.tile_pool(name="pci", bufs=2, space="PSUM") as pci_pool:

            for b in range(BL):
                hn_tiles = []
                twh = sm_pool.tile([128, 16], F32, name=f"twh{b}", tag="twh")
                for i in range(NT):
                    hn = hnat_pool.tile([128, 256], BF16, name=f"hn{b}_{i}", tag="hn")
                    hn_tiles.append(hn)
                    nc.sync.dma_start(hn[:, :], h_d[b, bass.ts(i, 128), :])
                    # transpose both d-halves: (128t,128d) -> (128d,128t)
                    pt0 = ptr_pool.tile([128, 128], BF16, name=f"pt0_{b}_{i}", tag="pt")
                    pt1 = ptr_pool.tile([128, 128], BF16, name=f"pt1_{b}_{i}", tag="pt")
                    nc.tensor.transpose(pt0[:, :], hn[:, 0:128], ident[:, :])
                    nc.tensor.transpose(pt1[:, :], hn[:, 128:256], ident[:, :])
                    ht0 = ht_pool.tile([128, 128], BF16, name=f"ht0_{b}_{i}", tag="ht0")
                    ht1 = ht_pool.tile([128, 128], BF16, name=f"ht1_{b}_{i}", tag="ht1")
                    nc.vector.tensor_copy(ht0[:, :], pt0[:, :])
                    nc.scalar.copy(ht1[:, :], pt1[:, :])
                    if i == 0:
                        nc.vector.tensor_copy(h0T[:, b:b + 1], ht0[:, 0:1])
                        nc.vector.tensor_copy(h0T[:, 8 + b:8 + b + 1], ht1[:, 0:1])
                    # wh = h @ W_h_a.T for this chunk: (128t, 256h)
                    pw = pwh_pool.tile([128, 256], F32, name=f"pw{b}_{i}", tag="pw")
                    nc.tensor.matmul(pw[:, :], ht0[:, :], whaT[:, 0:256],
                                     start=True, stop=False)
                    nc.tensor.matmul(pw[:, :], ht1[:, :], whaT[:, 256:512],
                                     start=False, stop=True)
                    th = sba_pool.tile([128, 256], BF16, name=f"th{b}_{i}", tag="th")
                    nc.scalar.activation(th[:, :], pw[:, :], AF.Tanh)
                    tw = sba_pool.tile([128, 256], BF16, name=f"tw{b}_{i}", tag="tw")
                    nc.vector.tensor_mul(tw[:, :], th[:, :], wa1r[:, :])
                    nc.vector.reduce_sum(twh[:, i:i + 1], tw[:, :], axis=AX.X)

                # e = exp(twh) (unnormalized) via exp(x) = (1+t)/(1-t),
                # t = tanh(x/2): the tanh table is ~100x more accurate than
                # the exp table (act_info err 4 vs 400; native Exp measured
                # 3.9e-2 end-to-end rel err vs 2e-3 with the identity).
                tt = sm_pool.tile([128, 16], F32, name=f"tt{b}", tag="tt")
                nc.scalar.activation(tt[:, :], twh[:, :], AF.Tanh, scale=0.5)
                uu = sm_pool.tile([128, 16], F32, name=f"uu{b}", tag="uu")
                nc.vector.tensor_scalar_add(uu[:, :], tt[:, :], 1.0)
                ww = sm_pool.tile([128, 16], F32, name=f"ww{b}", tag="ww")
                nc.vector.tensor_scalar(ww[:, :], tt[:, :], -1.0, 1.0,
                                        OP.mult, OP.add)
                rw = sm_pool.tile([128, 16], F32, name=f"rw{b}", tag="rw")
                nc.vector.reciprocal(rw[:, :], ww[:, :])
                ee = sm_pool.tile([128, 16], BF16, name=f"ee{b}", tag="ee")
                nc.vector.tensor_mul(ee[:, :], uu[:, :], rw[:, :])
                nc.vector.reduce_sum(partials[:, b:b + 1], ee[:, :], axis=AX.X)
                # unnormalized ci: (1,256) psum accumulated over chunks
                pci = pci_pool.tile([1, 256], F32, name=f"pci{b}", tag="pci")
                for i in range(NT):
                    nc.tensor.matmul(pci[:, :], ee[:, i:i + 1], hn_tiles[i][:, :],
                                     start=(i == 0), stop=(i == NT - 1))
                # route the (1,256) ci row into columns of (128,8) psum tiles
                cis = sm_pool.tile([1, 256], BF16, name=f"cis{b}", tag="cis")
                nc.vector.tensor_copy(cis[:, :], pci[:, :])
                nc.tensor.matmul(pciT0[:, b:b + 1], cis[0:1, 0:128],
                                 onescol[0:1, 0:1], start=True, stop=True)
                nc.tensor.matmul(pciT1[:, b:b + 1], cis[0:1, 128:256],
                                 onescol[0:1, 0:1], start=True, stop=True)

          # ---- phase A wrap-up ----
          with tc.tile_pool(name="wrap", bufs=2) as wr_pool, \
               tc.tile_pool(name="pwr", bufs=1, space="PSUM") as pwr_pool:
              # S_b = sum over partitions of partials[:, b] via ones matmul
              pb16 = wr_pool.tile([128, 8], BF16, name="pb16", tag="pb16")
              nc.vector.tensor_copy(pb16[:, :], partials[:, :])
              psums = pwr_pool.tile([1, 8], F32, name="psums", tag="psums")
              nc.tensor.matmul(psums[:, :], onescol[:, :], pb16[:, :],
                               start=True, stop=True)
              rS = wr_pool.tile([1, 8], F32, name="rS", tag="rS")
              nc.vector.reciprocal(rS[:, :], psums[:, :])
              rSb = wr_pool.tile([1, 8], BF16, name="rSb", tag="rSb")
              nc.vector.tensor_copy(rSb[:, :], rS[:, :])
              prS = pwr_pool.tile([128, 8], F32, name="prS", tag="prS")
              nc.tensor.matmul(prS[:, :], onesrow[:, :], rSb[:, :],
                               start=True, stop=True)
              rSs = wr_pool.tile([128, 8], F32, name="rSs", tag="rSs")
              nc.vector.tensor_copy(rSs[:, :], prS[:, :])
              # normalize ci columns -> ciT (128, 16) bf16
              # (DVE tensor_tensor can read at most one PSUM operand)
              nc.vector.tensor_mul(ciT[:, 0:8], pciT0[:, :], rSs[:, :])
              nc.vector.tensor_mul(ciT[:, 8:16], pciT1[:, :], rSs[:, :])
              # s0T = tanh(W_init @ h0 + b_init) in T layout
              for j in range(2):
                  ps0 = pwr_pool.tile([128, 8], F32, name=f"ps0{j}", tag="ps0")
                  nc.tensor.matmul(ps0[:, :], winitT[:, j * 128:j * 128 + 128],
                                   h0T[:, 0:8], start=True, stop=False)
                  nc.tensor.matmul(ps0[:, :], winitT[:, 256 + j * 128:256 + j * 128 + 128],
                                   h0T[:, 8:16], start=False, stop=True)
                  nc.scalar.activation(s0T[:, bass.ts(j, 8)], ps0[:, :], AF.Tanh,
                                       bias=binit[:, j:j + 1])
              # constgT = ci @ Wg_ci.T + biases (T layout, 6 f-tiles)
              for jj in range(6):
                  pcg = pwr_pool.tile([128, 8], F32, name=f"pcg{jj}", tag="pcg")
                  nc.tensor.matmul(pcg[:, :], wgciT[:, jj * 128:jj * 128 + 128],
                                   ciT[:, 0:8], start=True, stop=False)
                  nc.tensor.matmul(pcg[:, :], wgciT[:, 768 + jj * 128:768 + jj * 128 + 128],
                                   ciT[:, 8:16], start=False, stop=True)
                  nc.scalar.activation(constgT[:, bass.ts(jj, 8)], pcg[:, :],
                                       AF.Identity, bias=biasgT[:, jj:jj + 1])
              pcr = pwr_pool.tile([128, 8], F32, name="pcr", tag="pcr")
              nc.tensor.matmul(pcr[:, :], wrciT[:, 0:128], ciT[:, 0:8],
                               start=True, stop=False)
              nc.tensor.matmul(pcr[:, :], wrciT[:, 128:256], ciT[:, 8:16],
                               start=False, stop=True)
              nc.scalar.activation(constrT[:, :], pcr[:, :], AF.Identity,
                                   bias=biasrT[:, :])

        # ================= Phase B =================
        # PSUM accumulation-group discipline: start_tensor_calc=True wipes
        # the has-written state for the WHOLE bank, so a region must never
        # be accumulated into after another group opened in its bank.
        # Every region below is written by one CONSECUTIVE group and only
        # read afterwards.  Layout within the per-step (128,128) f32 bank:
        #   [0:32)   przs  rz gates, s contribution
        #   [32:48)  pnis  gh_n (s contribution)
        #   [48:80)  pyrz  rz gates, y contribution
        #   [80:96)  pyn   i_n y contribution
        #   [96:104) prn   RNN pre-activation
        #   [104:112) pbb  1/S broadcast   row0 [112:120) ps: sum(ey)
        outv = out_all.rearrange("p (b t) -> p b t", t=L)
        with tc.tile_pool(name="pb", bufs=4, space="PSUM") as pb_pool, \
             tc.tile_pool(name="sbb", bufs=4) as sbb_pool, \
             tc.tile_pool(name="stp", bufs=3) as st_pool:

            sT = s0T
            yT = None
            pend = None  # (ey tile, psum tile, t) awaiting softmax tail

            for t in range(L):
                pbt = pb_pool.tile([128, 128], F32, name=f"pbt{t}", tag="pb")
                # --- gate matmuls, s contributions (closed groups) ---
                for jj in range(4):
                    sl = pbt[:, jj * 8:jj * 8 + 8]
                    nc.tensor.matmul(sl, wgsT[:, jj * 128:jj * 128 + 128],
                                     sT[:, 0:8], start=True, stop=False)
                    nc.tensor.matmul(sl, wgsT[:, 768 + jj * 128:768 + jj * 128 + 128],
                                     sT[:, 8:16], start=False, stop=True)
                for jj in range(2):
                    sl = pbt[:, 32 + jj * 8:32 + jj * 8 + 8]
                    nc.tensor.matmul(sl, wgsT[:, (4 + jj) * 128:(4 + jj) * 128 + 128],
                                     sT[:, 0:8], start=True, stop=False)
                    nc.tensor.matmul(sl, wgsT[:, 768 + (4 + jj) * 128:768 + (4 + jj) * 128 + 128],
                                     sT[:, 8:16], start=False, stop=True)

                # --- previous step's softmax tail ---
                rzin = sbb_pool.tile([128, 32], F32, name=f"rzin{t}", tag="rzin")
                if pend is not None:
                    ey_p, pbt_p, t_p = pend
                    ps = pbt_p[0:1, 112:120]
                    nc.tensor.matmul(ps, onescol[:, :], ey_p[:, :],
                                     start=True, stop=True)
                    rs = sbb_pool.tile([1, 8], F32, name=f"rs{t}", tag="rs")
                    nc.vector.reciprocal(rs[:, :], ps)
                    rsb = sbb_pool.tile([1, 8], BF16, name=f"rsb{t}", tag="rsb")
                    nc.vector.tensor_copy(rsb[:, :], rs[:, :])
                    pbb = pbt_p[:, 104:112]
                    nc.tensor.matmul(pbb, onesrow[:, :], rsb[:, :],
                                     start=True, stop=True)
                    # rzin only needs przs -- emit before yout to overlap
                    nc.vector.tensor_add(rzin[:, :], pbt[:, 0:32],
                                         constgT[:, 0:32])
                    nc.vector.tensor_mul(outv[:, :, t_p], ey_p[:, :], pbb)
                    yT = outv[:, :, t_p]
                    pend = None
                else:
                    nc.vector.tensor_add(rzin[:, :], pbt[:, 0:32],
                                         constgT[:, 0:32])

                # --- gate matmuls, y contributions (own closed groups) ---
                if yT is not None:
                    for jj in range(4):
                        nc.tensor.matmul(pbt[:, 48 + jj * 8:48 + jj * 8 + 8],
                                         wgyT[:, jj * 128:jj * 128 + 128],
                                         yT, start=True, stop=True)
                    for jj in range(2):
                        nc.tensor.matmul(pbt[:, 80 + jj * 8:80 + jj * 8 + 8],
                                         wgyT[:, (4 + jj) * 128:(4 + jj) * 128 + 128],
                                         yT, start=True, stop=True)

                # --- GRU elementwise (T layout) ---
                if yT is not None:
                    rzin2 = sbb_pool.tile([128, 32], F32, name=f"rzin2{t}", tag="rzin2")
                    nc.vector.tensor_add(rzin2[:, :], rzin[:, :], pbt[:, 48:80])
                else:
                    rzin2 = rzin
                # split r/z: r gates the n-path (critical), z only at d2 --
                # sig_z runs on ACT in parallel with the DVE n-path ops
                sig = sbb_pool.tile([128, 32], F32, name=f"sig{t}", tag="sig")
                nc.scalar.activation(sig[:, 0:16], rzin2[:, 0:16], AF.Sigmoid)
                nc.scalar.activation(sig[:, 16:32], rzin2[:, 16:32], AF.Sigmoid)
                ns = sbb_pool.tile([128, 16], F32, name=f"ns{t}", tag="ns")
                if yT is None:
                    rh = sbb_pool.tile([128, 16], F32, name=f"rh{t}", tag="rh")
                    nc.vector.tensor_mul(rh[:, :], sig[:, 0:16], pbt[:, 32:48])
                    nc.vector.tensor_add(ns[:, :], rh[:, :], constgT[:, 32:48])
                else:
                    nin = sbb_pool.tile([128, 16], F32, name=f"nin{t}", tag="nin")
                    nc.vector.tensor_add(nin[:, :], pbt[:, 80:96], constgT[:, 32:48])
                    rh = sbb_pool.tile([128, 16], F32, name=f"rh{t}", tag="rh")
                    nc.vector.tensor_mul(rh[:, :], sig[:, 0:16], pbt[:, 32:48])
                    nc.vector.tensor_add(ns[:, :], rh[:, :], nin[:, :])
                nn = sbb_pool.tile([128, 16], BF16, name=f"nn{t}", tag="nn")
                nc.scalar.activation(nn[:, :], ns[:, :], AF.Tanh)
                d1 = sbb_pool.tile([128, 16], BF16, name=f"d1_{t}", tag="d1")
                nc.vector.tensor_sub(d1[:, :], sT[:, :], nn[:, :])
                d2 = sbb_pool.tile([128, 16], BF16, name=f"d2_{t}", tag="d2")
                nc.vector.tensor_mul(d2[:, :], sig[:, 16:32], d1[:, :])
                sTn = st_pool.tile([128, 16], BF16, name=f"sT{t}", tag="sT")
                nc.vector.tensor_add(sTn[:, :], nn[:, :], d2[:, :])

                # --- RNN cell + exp (one consecutive group) ---
                prn = pbt[:, 96:104]
                nc.tensor.matmul(prn, wrsT[:, 0:128], sTn[:, 0:8],
                                 start=True, stop=False)
                nc.tensor.matmul(prn, wrsT[:, 128:256], sTn[:, 8:16],
                                 start=False, stop=(yT is None))
                if yT is not None:
                    nc.tensor.matmul(prn, wryT[:, :], yT,
                                     start=False, stop=True)
                rin = sbb_pool.tile([128, 8], F32, name=f"rin{t}", tag="rin")
                nc.vector.tensor_add(rin[:, :], prn, constrT[:, :])
                vv = sbb_pool.tile([128, 8], F32, name=f"vv{t}", tag="vv")
                nc.scalar.activation(vv[:, :], rin[:, :], AF.Tanh)
                # exp via tanh identity (exp table too inaccurate)
                tv = sbb_pool.tile([128, 8], F32, name=f"tv{t}", tag="tv")
                nc.scalar.activation(tv[:, :], vv[:, :], AF.Tanh, scale=0.5)
                u2 = sbb_pool.tile([128, 8], F32, name=f"u2_{t}", tag="u2")
                nc.vector.tensor_scalar_add(u2[:, :], tv[:, :], 1.0)
                w2 = sbb_pool.tile([128, 8], F32, name=f"w2_{t}", tag="w2")
                nc.vector.tensor_scalar(w2[:, :], tv[:, :], -1.0, 1.0,
                                        OP.mult, OP.add)
                rw2 = sbb_pool.tile([128, 8], F32, name=f"rw2_{t}", tag="rw2")
                nc.vector.reciprocal(rw2[:, :], w2[:, :])
                ey = sbb_pool.tile([128, 8], BF16, name=f"ey{t}", tag="ey")
                nc.vector.tensor_mul(ey[:, :], u2[:, :], rw2[:, :])

                pend = (ey, pbt, t)
                sT = sTn

            # final step's softmax tail
            ey_p, pbt_p, t_p = pend
            ps = pbt_p[0:1, 112:120]
            nc.tensor.matmul(ps, onescol[:, :], ey_p[:, :], start=True, stop=True)
            rs = sbb_pool.tile([1, 8], F32, name="rsF", tag="rs")
            nc.vector.reciprocal(rs[:, :], ps)
            rsb = sbb_pool.tile([1, 8], BF16, name="rsbF", tag="rsb")
            nc.vector.tensor_copy(rsb[:, :], rs[:, :])
            pbb = pbt_p[:, 104:112]
            nc.tensor.matmul(pbb, onesrow[:, :], rsb[:, :], start=True, stop=True)
            nc.vector.tensor_mul(outv[:, :, t_p], ey_p[:, :], pbb)

        nc.sync.dma_start(out_d[:, :], out_all[:, :])

    nc.compile()
    return nc


def _pack_weights(inputs):
    f = np.float32
    bf = ml_dtypes.bfloat16
    W_h_a = np.asarray(inputs["W_h_a"], f)
    W_a = np.asarray(inputs["W_a"], f)
    W_init = np.asarray(inputs["W_init"], f)
    b_init = np.asarray(inputs["b_init"], f)
    W_ih_g = np.asarray(inputs["W_ih_g"], f)
    W_hh_g = np.asarray(inputs["W_hh_g"], f)
    b_ih_g = np.asarray(inputs["b_ih_g"], f)
    b_hh_g = np.asarray(inputs["b_hh_g"], f)
    W_ih_r = np.asarray(inputs["W_ih_r"], f)
    W_hh_r = np.asarray(inputs["W_hh_r"], f)
    b_ih_r = np.asarray(inputs["b_ih_r"], f)
    b_hh_r = np.asarray(inputs["b_hh_r"], f)

    assert not np.any(b_hh_g[512:]), "nonzero b_hh_g n-part not supported"

    def split2(m):  # (256, X) -> (128, 2X), k-chunks side by side
        return np.concatenate([m[0:128], m[128:256]], axis=1)

    wk = {}
    wk["whaT"] = split2(W_h_a.T).astype(bf)
    wk["wa1r"] = np.tile(W_a[0, :256][None, :], (128, 1)).astype(bf)
    wk["winitT"] = split2(W_init.T).astype(bf)
    wk["binit"] = np.ascontiguousarray(b_init.reshape(2, 128).T)
    wk["wgsT"] = split2(W_hh_g.T).astype(bf)
    wk["wgyT"] = W_ih_g[:, 256:384].T.astype(bf)
    wk["wgciT"] = split2(W_ih_g[:, 0:256].T).astype(bf)
    wk["wrsT"] = split2(W_ih_r[:, 256:512].T).astype(bf)
    wk["wryT"] = W_hh_r.T.astype(bf)
    wk["wrciT"] = split2(W_ih_r[:, 0:256].T).astype(bf)
    bias_g = b_ih_g + np.concatenate([b_hh_g[:512], np.zeros(256, f)])
    wk["biasgT"] = np.ascontiguousarray(bias_g.reshape(6, 128).T)
    wk["biasrT"] = np.ascontiguousarray((b_ih_r + b_hh_r).reshape(128, 1))
    wk["ident"] = np.eye(128, dtype=bf)
    wk["onescol"] = np.ones((128, 1), bf)
    wk["onesrow"] = np.ones((1, 128), bf)
    return {k: np.ascontiguousarray(v) for k, v in wk.items()}


def run(inputs, trace=False):
    from concourse import bass_utils

    assert int(inputs["out_len"]) == L
    if "nc" not in _CACHE:
        _CACHE["nc"] = _build_program()
    nc = _CACHE["nc"]

    wk = _pack_weights(inputs)
    h = np.asarray(inputs["h"], np.float32).astype(ml_dtypes.bfloat16)
    in_maps = []
    for c in range(NC):
        m = dict(wk)
        m["h"] = np.ascontiguousarray(h[c * BL:(c + 1) * BL])
        in_maps.append(m)

    res = bass_utils.run_bass_kernel_spmd(
        nc, in_maps, core_ids=list(range(NC)), trace=trace)

    out = np.empty((B, L, DO), np.float32)
    for c in range(NC):
        r = np.asarray(res.results[c]["out"]).astype(np.float32)
        r = r.reshape(128, BL, L)
        out[c * BL:(c + 1) * BL] = r.transpose(1, 2, 0)
    return out, res


def kernel(**inputs):
    out, _ = run(inputs, trace=False)
    return out


# revision 14
# speedup vs baseline: 1.0179x; 1.0179x over previous
"""Trainium2 Bass kernel for nn_AttentionDecoder (bf16 rewrite).

Key insight (from the reference): the per-step attention score adds a
per-batch scalar (sa) to every element of the row before softmax;
softmax is shift-invariant, so the attention weights -- and the context
vector ci -- are identical for all 64 decode steps.  The computation
collapses to:

  Phase A (streams h once):
     twh[b,t] = sum_h tanh(h[b,t,:] @ W_h_a.T)[h] * wa1[h]
     e        = exp(twh)            (unnormalized; |twh| <= ~10)
     ci[b,:]  = (e @ h[b]) / sum(e)
     s0       = tanh(h[:,0,:] @ W_init.T + b_init)
  Phase B (64 sequential GRU+RNN steps, batch=8 per core):
     si = GRU(cat(ci,y), s);  yi = softmax(tanh(RNN(cat(ci,si), y)))

Perf notes vs the fp32 baseline (1.14 ms):
  * All matmuls in bf16: fp32 matmuls run twice (fp32_mode=LOW/HIGH, two
    LDWEIGHTS+MATMUL pairs each) and disable FWL fast weight load.  bf16
    is single-pass with FWL -- production-measured ~81 ns/MM at N=128.
  * exp is computed via exp(x) = (1+tanh(x/2))/(1-tanh(x/2)) -- the HW
    exp table is ~100x less accurate than the tanh table (act_info err
    400 vs 4; native Exp measured 3.9e-2 end-to-end rel err vs 6e-3).
    GRU gates use the native Sigmoid table (same set as tanh).
  * Zero GpSimd use (baseline lost ~190us to gpsimd semaphore overhead).
    Partition reductions/broadcasts use k=1 matmuls against ones.
  * Phase B software-pipelined: step t's s-dependent gate matmuls are
    emitted before step t-1's softmax tail so the PE stays busy.
  * PSUM accumulation groups are never interleaved with other groups in
    the same bank: start_tensor_calc=True wipes the bank's has-written
    state, silently dropping earlier partial sums (measured, not in the
    docs).  y contributions get their own single-matmul regions and are
    merged on the vector engine instead.

Sharding: data-parallel over batch, 8 batches per core, weights
replicated; h is cast to bf16 host-side (tolerance is 2e-2).
"""

import numpy as np
import ml_dtypes

B, T, D, H, DO, L = 64, 2048, 256, 256, 128, 64
NC = 8           # cores
BL = B // NC     # batches per core = 8
NT = T // 128    # 16 t-chunks

_CACHE = {}


def _build_program():
    import concourse.bass as bass
    import concourse.bacc as bacc
    import concourse.mybir as mybir
    import concourse.tile as tile

    dt = mybir.dt
    F32 = dt.float32
    BF16 = dt.bfloat16
    AF = mybir.ActivationFunctionType
    OP = mybir.AluOpType
    AX = mybir.AxisListType

    nc = bacc.Bacc("TRN2", target_bir_lowering=False, debug=False, num_devices=NC)

    # ---- DRAM I/O ------------------------------------------------------
    h_d = nc.dram_tensor("h", (BL, T, D), BF16, kind="ExternalInput").ap()
    whaT_d = nc.dram_tensor("whaT", (128, 512), BF16, kind="ExternalInput").ap()
    wa1r_d = nc.dram_tensor("wa1r", (128, 256), BF16, kind="ExternalInput").ap()
    winitT_d = nc.dram_tensor("winitT", (128, 512), BF16, kind="ExternalInput").ap()
    binit_d = nc.dram_tensor("binit", (128, 2), F32, kind="ExternalInput").ap()
    wgsT_d = nc.dram_tensor("wgsT", (128, 1536), BF16, kind="ExternalInput").ap()
    wgyT_d = nc.dram_tensor("wgyT", (128, 768), BF16, kind="ExternalInput").ap()
    wgciT_d = nc.dram_tensor("wgciT", (128, 1536), BF16, kind="ExternalInput").ap()
    wrsT_d = nc.dram_tensor("wrsT", (128, 256), BF16, kind="ExternalInput").ap()
    wryT_d = nc.dram_tensor("wryT", (128, 128), BF16, kind="ExternalInput").ap()
    wrciT_d = nc.dram_tensor("wrciT", (128, 256), BF16, kind="ExternalInput").ap()
    biasgT_d = nc.dram_tensor("biasgT", (128, 6), F32, kind="ExternalInput").ap()
    biasrT_d = nc.dram_tensor("biasrT", (128, 1), F32, kind="ExternalInput").ap()
    ident_d = nc.dram_tensor("ident", (128, 128), BF16, kind="ExternalInput").ap()
    onescol_d = nc.dram_tensor("onescol", (128, 1), BF16, kind="ExternalInput").ap()
    onesrow_d = nc.dram_tensor("onesrow", (1, 128), BF16, kind="ExternalInput").ap()
    out_d = nc.dram_tensor("out", (128, BL * L), BF16, kind="ExternalOutput").ap()

    # ---- persistent SBUF ----------------------------------------------
    whaT = nc.alloc_sbuf_tensor("whaT_sb", [128, 512], BF16).ap()
    wa1r = nc.alloc_sbuf_tensor("wa1r_sb", [128, 256], BF16).ap()
    winitT = nc.alloc_sbuf_tensor("winitT_sb", [128, 512], BF16).ap()
    binit = nc.alloc_sbuf_tensor("binit_sb", [128, 2], F32).ap()
    wgsT = nc.alloc_sbuf_tensor("wgsT_sb", [128, 1536], BF16).ap()
    wgyT = nc.alloc_sbuf_tensor("wgyT_sb", [128, 768], BF16).ap()
    wgciT = nc.alloc_sbuf_tensor("wgciT_sb", [128, 1536], BF16).ap()
    wrsT = nc.alloc_sbuf_tensor("wrsT_sb", [128, 256], BF16).ap()
    wryT = nc.alloc_sbuf_tensor("wryT_sb", [128, 128], BF16).ap()
    wrciT = nc.alloc_sbuf_tensor("wrciT_sb", [128, 256], BF16).ap()
    biasgT = nc.alloc_sbuf_tensor("biasgT_sb", [128, 6], F32).ap()
    biasrT = nc.alloc_sbuf_tensor("biasrT_sb", [128, 1], F32).ap()
    ident = nc.alloc_sbuf_tensor("ident_sb", [128, 128], BF16).ap()
    onescol = nc.alloc_sbuf_tensor("onescol_sb", [128, 1], BF16).ap()
    onesrow = nc.alloc_sbuf_tensor("onesrow_sb", [1, 128], BF16).ap()

    h0T = nc.alloc_sbuf_tensor("h0T", [128, 16], BF16).ap()      # h[:,0,:] cols c*8+b
    partials = nc.alloc_sbuf_tensor("partials", [128, 8], F32).ap()
    ciT = nc.alloc_sbuf_tensor("ciT", [128, 16], BF16).ap()      # cols c*8+b
    s0T = nc.alloc_sbuf_tensor("s0T", [128, 16], BF16).ap()
    constgT = nc.alloc_sbuf_tensor("constgT", [128, 48], F32).ap()
    constrT = nc.alloc_sbuf_tensor("constrT", [128, 8], F32).ap()
    out_all = nc.alloc_sbuf_tensor("out_all", [128, BL * L], BF16).ap()

    with tile.TileContext(nc) as tc:
        # weight loads
        for sb, dr in [(whaT, whaT_d), (wa1r, wa1r_d), (winitT, winitT_d),
                       (binit, binit_d), (wgsT, wgsT_d), (wgyT, wgyT_d),
                       (wgciT, wgciT_d), (wrsT, wrsT_d), (wryT, wryT_d),
                       (wrciT, wrciT_d), (biasgT, biasgT_d), (biasrT, biasrT_d),
                       (ident, ident_d), (onescol, onescol_d),
                       (onesrow, onesrow_d)]:
            nc.sync.dma_start(sb[:, :], dr[:, :])

        # ================= Phase A =================
        with tc.tile_pool(name="pcit", bufs=1, space="PSUM") as pcit_pool:
          pciT0 = pcit_pool.tile([128, 8], F32, name="pciT0", tag="pciT0")
          pciT1 = pcit_pool.tile([128, 8], F32, name="pciT1", tag="pciT1")
          with tc.tile_pool(name="hnat", bufs=24) as hnat_pool, \
             tc.tile_pool(name="hts", bufs=6) as ht_pool, \
             tc.tile_pool(name="sba", bufs=3) as sba_pool, \
             tc.tile_pool(name="smalla", bufs=3) as sm_pool, \
             tc.tile_pool(name="ptr", bufs=2, space="PSUM") as ptr_pool, \
             tc.tile_pool(name="pwh", bufs=2, space="PSUM") as pwh_pool, \
             tc.tile_pool(name="pci", bufs=2, space="PSUM") as pci_pool:

            for b in range(BL):
                hn_tiles = []
                twh = sm_pool.tile([128, 16], F32, name=f"twh{b}", tag="twh")
                for i in range(NT):
                    hn = hnat_pool.tile([128, 256], BF16, name=f"hn{b}_{i}", tag="hn")
                    hn_tiles.append(hn)
                    nc.sync.dma_start(hn[:, :], h_d[b, bass.ts(i, 128), :])
                    # transpose both d-halves: (128t,128d) -> (128d,128t)
                    pt0 = ptr_pool.tile([128, 128], BF16, name=f"pt0_{b}_{i}", tag="pt")
                    pt1 = ptr_pool.tile([128, 128], BF16, name=f"pt1_{b}_{i}", tag="pt")
                    nc.tensor.transpose(pt0[:, :], hn[:, 0:128], ident[:, :])
                    nc.tensor.transpose(pt1[:, :], hn[:, 128:256], ident[:, :])
                    ht0 = ht_pool.tile([128, 128], BF16, name=f"ht0_{b}_{i}", tag="ht0")
                    ht1 = ht_pool.tile([128, 128], BF16, name=f"ht1_{b}_{i}", tag="ht1")
                    nc.vector.tensor_copy(ht0[:, :], pt0[:, :])
                    nc.scalar.copy(ht1[:, :], pt1[:, :])
                    if i == 0:
                        nc.vector.tensor_copy(h0T[:, b:b + 1], ht0[:, 0:1])
                        nc.vector.tensor_copy(h0T[:, 8 + b:8 + b + 1], ht1[:, 0:1])
                    # wh = h @ W_h_a.T for this chunk: (128t, 256h)
                    pw = pwh_pool.tile([128, 256], F32, name=f"pw{b}_{i}", tag="pw")
                    nc.tensor.matmul(pw[:, :], ht0[:, :], whaT[:, 0:256],
                                     start=True, stop=False)
                    nc.tensor.matmul(pw[:, :], ht1[:, :], whaT[:, 256:512],
                                     start=False, stop=True)
                    th = sba_pool.tile([128, 256], BF16, name=f"th{b}_{i}", tag="th")
                    nc.scalar.activation(th[:, :], pw[:, :], AF.Tanh)
                    tw = sba_pool.tile([128, 256], BF16, name=f"tw{b}_{i}", tag="tw")
                    nc.vector.tensor_mul(tw[:, :], th[:, :], wa1r[:, :])
                    nc.vector.reduce_sum(twh[:, i:i + 1], tw[:, :], axis=AX.X)

                # e = exp(twh) (unnormalized) via exp(x) = (1+t)/(1-t),
                # t = tanh(x/2): the tanh table is ~100x more accurate than
                # the exp table (act_info err 4 vs 400; native Exp measured
                # 3.9e-2 end-to-end rel err vs 2e-3 with the identity).
                tt = sm_pool.tile([128, 16], F32, name=f"tt{b}", tag="tt")
                nc.scalar.activation(tt[:, :], twh[:, :], AF.Tanh, scale=0.5)
                uu = sm_pool.tile([128, 16], F32, name=f"uu{b}", tag="uu")
                nc.vector.tensor_scalar_add(uu[:, :], tt[:, :], 1.0)
                ww = sm_pool.tile([128, 16], F32, name=f"ww{b}", tag="ww")
                nc.vector.tensor_scalar(ww[:, :], tt[:, :], -1.0, 1.0,
                                        OP.mult, OP.add)
                rw = sm_pool.tile([128, 16], F32, name=f"rw{b}", tag="rw")
                nc.vector.reciprocal(rw[:, :], ww[:, :])
                ee = sm_pool.tile([128, 16], BF16, name=f"ee{b}", tag="ee")
                nc.vector.tensor_mul(ee[:, :], uu[:, :], rw[:, :])
                nc.vector.reduce_sum(partials[:, b:b + 1], ee[:, :], axis=AX.X)
                # unnormalized ci: (1,256) psum accumulated over chunks
                pci = pci_pool.tile([1, 256], F32, name=f"pci{b}", tag="pci")
                for i in range(NT):
                    nc.tensor.matmul(pci[:, :], ee[:, i:i + 1], hn_tiles[i][:, :],
                                     start=(i == 0), stop=(i == NT - 1))
                # route the (1,256) ci row into columns of (128,8) psum tiles
                cis = sm_pool.tile([1, 256], BF16, name=f"cis{b}", tag="cis")
                nc.vector.tensor_copy(cis[:, :], pci[:, :])
                nc.tensor.matmul(pciT0[:, b:b + 1], cis[0:1, 0:128],
                                 onescol[0:1, 0:1], start=True, stop=True)
                nc.tensor.matmul(pciT1[:, b:b + 1], cis[0:1, 128:256],
                                 onescol[0:1, 0:1], start=True, stop=True)

          # ---- phase A wrap-up ----
          with tc.tile_pool(name="wrap", bufs=2) as wr_pool, \
               tc.tile_pool(name="pwr", bufs=1, space="PSUM") as pwr_pool:
              # S_b = sum over partitions of partials[:, b] via ones matmul
              pb16 = wr_pool.tile([128, 8], BF16, name="pb16", tag="pb16")
              nc.vector.tensor_copy(pb16[:, :], partials[:, :])
              psums = pwr_pool.tile([1, 8], F32, name="psums", tag="psums")
              nc.tensor.matmul(psums[:, :], onescol[:, :], pb16[:, :],
                               start=True, stop=True)
              rS = wr_pool.tile([1, 8], F32, name="rS", tag="rS")
              nc.vector.reciprocal(rS[:, :], psums[:, :])
              rSb = wr_pool.tile([1, 8], BF16, name="rSb", tag="rSb")
              nc.vector.tensor_copy(rSb[:, :], rS[:, :])
              prS = pwr_pool.tile([128, 8], F32, name="prS", tag="prS")
              nc.tensor.matmul(prS[:, :], onesrow[:, :], rSb[:, :],
                               start=True, stop=True)
              rSs = wr_pool.tile([128, 8], F32, name="rSs", tag="rSs")
              nc.vector.tensor_copy(rSs[:, :], prS[:, :])
              # normalize ci columns -> ciT (128, 16) bf16
              # (DVE tensor_tensor can read at most one PSUM operand)
              nc.vector.tensor_mul(ciT[:, 0:8], pciT0[:, :], rSs[:, :])
              nc.vector.tensor_mul(ciT[:, 8:16], pciT1[:, :], rSs[:, :])
              # s0T = tanh(W_init @ h0 + b_init) in T layout
              for j in range(2):
                  ps0 = pwr_pool.tile([128, 8], F32, name=f"ps0{j}", tag="ps0")
                  nc.tensor.matmul(ps0[:, :], winitT[:, j * 128:j * 128 + 128],
                                   h0T[:, 0:8], start=True, stop=False)
                  nc.tensor.matmul(ps0[:, :], winitT[:, 256 + j * 128:256 + j * 128 + 128],
                                   h0T[:, 8:16], start=False, stop=True)
                  nc.scalar.activation(s0T[:, bass.ts(j, 8)], ps0[:, :], AF.Tanh,
                                       bias=binit[:, j:j + 1])
              # constgT = ci @ Wg_ci.T + biases (T layout, 6 f-tiles)
              for jj in range(6):
                  pcg = pwr_pool.tile([128, 8], F32, name=f"pcg{jj}", tag="pcg")
                  nc.tensor.matmul(pcg[:, :], wgciT[:, jj * 128:jj * 128 + 128],
                                   ciT[:, 0:8], start=True, stop=False)
                  nc.tensor.matmul(pcg[:, :], wgciT[:, 768 + jj * 128:768 + jj * 128 + 128],
                                   ciT[:, 8:16], start=False, stop=True)
                  nc.scalar.activation(constgT[:, bass.ts(jj, 8)], pcg[:, :],
                                       AF.Identity, bias=biasgT[:, jj:jj + 1])
              pcr = pwr_pool.tile([128, 8], F32, name="pcr", tag="pcr")
              nc.tensor.matmul(pcr[:, :], wrciT[:, 0:128], ciT[:, 0:8],
                               start=True, stop=False)
              nc.tensor.matmul(pcr[:, :], wrciT[:, 128:256], ciT[:, 8:16],
                               start=False, stop=True)
              nc.scalar.activation(constrT[:, :], pcr[:, :], AF.Identity,
                                   bias=biasrT[:, :])

        # ================= Phase B =================
        # PSUM accumulation-group discipline: start_tensor_calc=True wipes
        # the has-written state for the WHOLE bank, so a region must never
        # be accumulated into after another group opened in its bank.
        # Every region below is written by one CONSECUTIVE group and only
        # read afterwards.  Layout within the per-step (128,128) f32 bank:
        #   [0:32)   przs  rz gates, s contribution
        #   [32:48)  pnis  gh_n (s contribution)
        #   [48:80)  pyrz  rz gates, y contribution
        #   [80:96)  pyn   i_n y contribution
        #   [96:104) prn   RNN pre-activation
        #   [104:112) pbb  1/S broadcast   row0 [112:120) ps: sum(ey)
        outv = out_all.rearrange("p (b t) -> p b t", t=L)
        with tc.tile_pool(name="pb", bufs=4, space="PSUM") as pb_pool, \
             tc.tile_pool(name="sbb", bufs=4) as sbb_pool, \
             tc.tile_pool(name="stp", bufs=3) as st_pool:

            sT = s0T
            yT = None
            pend = None  # (ey tile, psum tile, t) awaiting softmax tail

            for t in range(L):
                pbt = pb_pool.tile([128, 128], F32, name=f"pbt{t}", tag="pb")
                # --- gate matmuls, s contributions (closed groups) ---
                for jj in range(4):
                    sl = pbt[:, jj * 8:jj * 8 + 8]
                    nc.tensor.matmul(sl, wgsT[:, jj * 128:jj * 128 + 128],
                                     sT[:, 0:8], start=True, stop=False)
                    nc.tensor.matmul(sl, wgsT[:, 768 + jj * 128:768 + jj * 128 + 128],
                                     sT[:, 8:16], start=False, stop=True)
                for jj in range(2):
                    sl = pbt[:, 32 + jj * 8:32 + jj * 8 + 8]
                    nc.tensor.matmul(sl, wgsT[:, (4 + jj) * 128:(4 + jj) * 128 + 128],
                                     sT[:, 0:8], start=True, stop=False)
                    nc.tensor.matmul(sl, wgsT[:, 768 + (4 + jj) * 128:768 + (4 + jj) * 128 + 128],
                                     sT[:, 8:16], start=False, stop=True)

                # --- previous step's softmax tail ---
                rzin = sbb_pool.tile([128, 32], F32, name=f"rzin{t}", tag="rzin")
                if pend is not None:
                    ey_p, pbt_p, t_p = pend
                    ps = pbt_p[0:1, 112:120]
                    nc.tensor.matmul(ps, onescol[:, :], ey_p[:, :],
                                     start=True, stop=True)
                    rs = sbb_pool.tile([1, 8], F32, name=f"rs{t}", tag="rs")
                    nc.vector.reciprocal(rs[:, :], ps)
                    rsb = sbb_pool.tile([1, 8], BF16, name=f"rsb{t}", tag="rsb")
                    nc.vector.tensor_copy(rsb[:, :], rs[:, :])
                    pbb = pbt_p[:, 104:112]
                    nc.tensor.matmul(pbb, onesrow[:, :], rsb[:, :],
                                     start=True, stop=True)
                    # rzin only needs przs -- emit before yout to overlap
                    nc.vector.tensor_add(rzin[:, :], pbt[:, 0:32],
                                         constgT[:, 0:32])
                    nc.vector.tensor_mul(outv[:, :, t_p], ey_p[:, :], pbb)
                    yT = outv[:, :, t_p]
                    pend = None
                else:
                    nc.vector.tensor_add(rzin[:, :], pbt[:, 0:32],
                                         constgT[:, 0:32])

                # --- gate matmuls, y contributions (own closed groups) ---
                if yT is not None:
                    for jj in range(4):
                        nc.tensor.matmul(pbt[:, 48 + jj * 8:48 + jj * 8 + 8],
                                         wgyT[:, jj * 128:jj * 128 + 128],
                                         yT, start=True, stop=True)
                    for jj in range(2):
                        nc.tensor.matmul(pbt[:, 80 + jj * 8:80 + jj * 8 + 8],
                                         wgyT[:, (4 + jj) * 128:(4 + jj) * 128 + 128],
                                         yT, start=True, stop=True)

                # --- GRU elementwise (T layout) ---
                if yT is not None:
                    rzin2 = sbb_pool.tile([128, 32], F32, name=f"rzin2{t}", tag="rzin2")
                    nc.vector.tensor_add(rzin2[:, :], rzin[:, :], pbt[:, 48:80])
                else:
                    rzin2 = rzin
                # split r/z: r gates the n-path (critical), z only at d2 --
                # sig_z runs on ACT in parallel with the DVE n-path ops
                sig = sbb_pool.tile([128, 32], F32, name=f"sig{t}", tag="sig")
                nc.scalar.activation(sig[:, 0:16], rzin2[:, 0:16], AF.Sigmoid)
                nc.scalar.activation(sig[:, 16:32], rzin2[:, 16:32], AF.Sigmoid)
                ns = sbb_pool.tile([128, 16], F32, name=f"ns{t}", tag="ns")
                if yT is None:
                    rh = sbb_pool.tile([128, 16], F32, name=f"rh{t}", tag="rh")
                    nc.vector.tensor_mul(rh[:, :], sig[:, 0:16], pbt[:, 32:48])
                    nc.vector.tensor_add(ns[:, :], rh[:, :], constgT[:, 32:48])
                else:
                    nin = sbb_pool.tile([128, 16], F32, name=f"nin{t}", tag="nin")
                    nc.vector.tensor_add(nin[:, :], pbt[:, 80:96], constgT[:, 32:48])
                    rh = sbb_pool.tile([128, 16], F32, name=f"rh{t}", tag="rh")
                    nc.vector.tensor_mul(rh[:, :], sig[:, 0:16], pbt[:, 32:48])
                    nc.vector.tensor_add(ns[:, :], rh[:, :], nin[:, :])
                nn = sbb_pool.tile([128, 16], BF16, name=f"nn{t}", tag="nn")
                nc.scalar.activation(nn[:, :], ns[:, :], AF.Tanh)
                d1 = sbb_pool.tile([128, 16], BF16, name=f"d1_{t}", tag="d1")
                nc.vector.tensor_sub(d1[:, :], sT[:, :], nn[:, :])
                d2 = sbb_pool.tile([128, 16], BF16, name=f"d2_{t}", tag="d2")
                nc.vector.tensor_mul(d2[:, :], sig[:, 16:32], d1[:, :])
                sTn = st_pool.tile([128, 16], BF16, name=f"sT{t}", tag="sT")
                nc.vector.tensor_add(sTn[:, :], nn[:, :], d2[:, :])

                # --- RNN cell + exp (one consecutive group) ---
                prn = pbt[:, 96:104]
                nc.tensor.matmul(prn, wrsT[:, 0:128], sTn[:, 0:8],
                                 start=True, stop=False)
                nc.tensor.matmul(prn, wrsT[:, 128:256], sTn[:, 8:16],
                                 start=False, stop=(yT is None))
                if yT is not None:
                    nc.tensor.matmul(prn, wryT[:, :], yT,
                                     start=False, stop=True)
                rin = sbb_pool.tile([128, 8], F32, name=f"rin{t}", tag="rin")
                nc.vector.tensor_add(rin[:, :], prn, constrT[:, :])
                vv = sbb_pool.tile([128, 8], F32, name=f"vv{t}", tag="vv")
                nc.scalar.activation(vv[:, :], rin[:, :], AF.Tanh)
                # exp via tanh identity (exp table too inaccurate)
                tv = sbb_pool.tile([128, 8], F32, name=f"tv{t}", tag="tv")
                nc.scalar.activation(tv[:, :], vv[:, :], AF.Tanh, scale=0.5)
                u2 = sbb_pool.tile([128, 8], F32, name=f"u2_{t}", tag="u2")
                nc.vector.tensor_scalar_add(u2[:, :], tv[:, :], 1.0)
                w2 = sbb_pool.tile([128, 8], F32, name=f"w2_{t}", tag="w2")
                nc.vector.tensor_scalar(w2[:, :], tv[:, :], -1.0, 1.0,
                                        OP.mult, OP.add)
                rw2 = sbb_pool.tile([128, 8], F32, name=f"rw2_{t}", tag="rw2")
                nc.vector.reciprocal(rw2[:, :], w2[:, :])
                ey = sbb_pool.tile([128, 8], BF16, name=f"ey{t}", tag="ey")
                nc.vector.tensor_mul(ey[:, :], u2[:, :], rw2[:, :])

                pend = (ey, pbt, t)
                sT = sTn

            # final step's softmax tail
            ey_p, pbt_p, t_p = pend
            ps = pbt_p[0:1, 112:120]
            nc.tensor.matmul(ps, onescol[:, :], ey_p[:, :], start=True, stop=True)
            rs = sbb_pool.tile([1, 8], F32, name="rsF", tag="rs")
            nc.vector.reciprocal(rs[:, :], ps)
            rsb = sbb_pool.tile([1, 8], BF16, name="rsbF", tag="rsb")
            nc.vector.tensor_copy(rsb[:, :], rs[:, :])
            pbb = pbt_p[:, 104:112]
            nc.tensor.matmul(pbb, onesrow[:, :], rsb[:, :], start=True, stop=True)
            nc.vector.tensor_mul(outv[:, :, t_p], ey_p[:, :], pbb)

        nc.sync.dma_start(out_d[:, :], out_all[:, :])

    nc.compile()
    return nc


def _pack_weights(inputs):
    f = np.float32
    bf = ml_dtypes.bfloat16
    W_h_a = np.asarray(inputs["W_h_a"], f)
    W_a = np.asarray(inputs["W_a"], f)
    W_init = np.asarray(inputs["W_init"], f)
    b_init = np.asarray(inputs["b_init"], f)
    W_ih_g = np.asarray(inputs["W_ih_g"], f)
    W_hh_g = np.asarray(inputs["W_hh_g"], f)
    b_ih_g = np.asarray(inputs["b_ih_g"], f)
    b_hh_g = np.asarray(inputs["b_hh_g"], f)
    W_ih_r = np.asarray(inputs["W_ih_r"], f)
    W_hh_r = np.asarray(inputs["W_hh_r"], f)
    b_ih_r = np.asarray(inputs["b_ih_r"], f)
    b_hh_r = np.asarray(inputs["b_hh_r"], f)

    assert not np.any(b_hh_g[512:]), "nonzero b_hh_g n-part not supported"

    def split2(m):  # (256, X) -> (128, 2X), k-chunks side by side
        return np.concatenate([m[0:128], m[128:256]], axis=1)

    wk = {}
    wk["whaT"] = split2(W_h_a.T).astype(bf)
    wk["wa1r"] = np.tile(W_a[0, :256][None, :], (128, 1)).astype(bf)
    wk["winitT"] = split2(W_init.T).astype(bf)
    wk["binit"] = np.ascontiguousarray(b_init.reshape(2, 128).T)
    wk["wgsT"] = split2(W_hh_g.T).astype(bf)
    wk["wgyT"] = W_ih_g[:, 256:384].T.astype(bf)
    wk["wgciT"] = split2(W_ih_g[:, 0:256].T).astype(bf)
    wk["wrsT"] = split2(W_ih_r[:, 256:512].T).astype(bf)
    wk["wryT"] = W_hh_r.T.astype(bf)
    wk["wrciT"] = split2(W_ih_r[:, 0:256].T).astype(bf)
    bias_g = b_ih_g + np.concatenate([b_hh_g[:512], np.zeros(256, f)])
    wk["biasgT"] = np.ascontiguousarray(bias_g.reshape(6, 128).T)
    wk["biasrT"] = np.ascontiguousarray((b_ih_r + b_hh_r).reshape(128, 1))
    wk["ident"] = np.eye(128, dtype=bf)
    wk["onescol"] = np.ones((128, 1), bf)
    wk["onesrow"] = np.ones((1, 128), bf)
    return {k: np.ascontiguousarray(v) for k, v in wk.items()}


def run(inputs, trace=False):
    from concourse import bass_utils

    assert int(inputs["out_len"]) == L
    if "nc" not in _CACHE:
        _CACHE["nc"] = _build_program()
    nc = _CACHE["nc"]

    wk = _pack_weights(inputs)
    h = np.asarray(inputs["h"], np.float32).astype(ml_dtypes.bfloat16)
    in_maps = []
    for c in range(NC):
        m = dict(wk)
        m["h"] = np.ascontiguousarray(h[c * BL:(c + 1) * BL])
        in_maps.append(m)

    res = bass_utils.run_bass_kernel_spmd(
        nc, in_maps, core_ids=list(range(NC)), trace=trace)

    out = np.empty((B, L, DO), np.float32)
    for c in range(NC):
        r = np.asarray(res.results[c]["out"]).astype(np.float32)
        r = r.reshape(128, BL, L)
        out[c * BL:(c + 1) * BL] = r.transpose(1, 2, 0)
    return out, res


def kernel(**inputs):
    out, _ = run(inputs, trace=False)
    return out


# revision 16
# speedup vs baseline: 1.0695x; 1.0507x over previous
"""Trainium2 Bass kernel for nn_AttentionDecoder (bf16 rewrite).

Key insight (from the reference): the per-step attention score adds a
per-batch scalar (sa) to every element of the row before softmax;
softmax is shift-invariant, so the attention weights -- and the context
vector ci -- are identical for all 64 decode steps.  The computation
collapses to:

  Phase A (streams h once):
     twh[b,t] = sum_h tanh(h[b,t,:] @ W_h_a.T)[h] * wa1[h]
     e        = exp(twh)            (unnormalized; |twh| <= ~10)
     ci[b,:]  = (e @ h[b]) / sum(e)
     s0       = tanh(h[:,0,:] @ W_init.T + b_init)
  Phase B (64 sequential GRU+RNN steps, batch=8 per core):
     si = GRU(cat(ci,y), s);  yi = softmax(tanh(RNN(cat(ci,si), y)))

Perf notes vs the fp32 baseline (1.14 ms):
  * All matmuls in bf16: fp32 matmuls run twice (fp32_mode=LOW/HIGH, two
    LDWEIGHTS+MATMUL pairs each) and disable FWL fast weight load.  bf16
    is single-pass with FWL -- production-measured ~81 ns/MM at N=128.
  * exp is computed via exp(x) = (1+tanh(x/2))/(1-tanh(x/2)) -- the HW
    exp table is ~100x less accurate than the tanh table (act_info err
    400 vs 4; native Exp measured 3.9e-2 end-to-end rel err vs 6e-3).
    GRU gates use the native Sigmoid table (same set as tanh).
  * Zero GpSimd use (baseline lost ~190us to gpsimd semaphore overhead).
    Partition reductions/broadcasts use k=1 matmuls against ones.
  * Phase B software-pipelined: step t's s-dependent gate matmuls are
    emitted before step t-1's softmax tail so the PE stays busy.
  * PSUM accumulation groups are never interleaved with other groups in
    the same bank: start_tensor_calc=True wipes the bank's has-written
    state, silently dropping earlier partial sums (measured, not in the
    docs).  y contributions get their own single-matmul regions and are
    merged on the vector engine instead.

Sharding: data-parallel over batch, 8 batches per core, weights
replicated; h is cast to bf16 host-side (tolerance is 2e-2).
"""

import numpy as np
import ml_dtypes

B, T, D, H, DO, L = 64, 2048, 256, 256, 128, 64
NC = 8           # cores
BL = B // NC     # batches per core = 8
NT = T // 128    # 16 t-chunks

_CACHE = {}


def _build_program():
    import concourse.bass as bass
    import concourse.bacc as bacc
    import concourse.mybir as mybir
    import concourse.tile as tile

    dt = mybir.dt
    F32 = dt.float32
    BF16 = dt.bfloat16
    AF = mybir.ActivationFunctionType
    OP = mybir.AluOpType
    AX = mybir.AxisListType

    nc = bacc.Bacc("TRN2", target_bir_lowering=False, debug=False, num_devices=NC)

    # ---- DRAM I/O ------------------------------------------------------
    h_d = nc.dram_tensor("h", (BL, T, D), BF16, kind="ExternalInput").ap()
    whaT_d = nc.dram_tensor("whaT", (128, 512), BF16, kind="ExternalInput").ap()
    wa1r_d = nc.dram_tensor("wa1r", (128, 256), BF16, kind="ExternalInput").ap()
    winitT_d = nc.dram_tensor("winitT", (128, 512), BF16, kind="ExternalInput").ap()
    binit_d = nc.dram_tensor("binit", (128, 2), F32, kind="ExternalInput").ap()
    wgsT_d = nc.dram_tensor("wgsT", (128, 1536), BF16, kind="ExternalInput").ap()
    wgyT_d = nc.dram_tensor("wgyT", (128, 768), BF16, kind="ExternalInput").ap()
    wgciT_d = nc.dram_tensor("wgciT", (128, 1536), BF16, kind="ExternalInput").ap()
    wrsT_d = nc.dram_tensor("wrsT", (128, 256), BF16, kind="ExternalInput").ap()
    wryT_d = nc.dram_tensor("wryT", (128, 128), BF16, kind="ExternalInput").ap()
    wrciT_d = nc.dram_tensor("wrciT", (128, 256), BF16, kind="ExternalInput").ap()
    biasgT_d = nc.dram_tensor("biasgT", (128, 6), F32, kind="ExternalInput").ap()
    biasrT_d = nc.dram_tensor("biasrT", (128, 1), F32, kind="ExternalInput").ap()
    ident_d = nc.dram_tensor("ident", (128, 128), BF16, kind="ExternalInput").ap()
    onescol_d = nc.dram_tensor("onescol", (128, 1), BF16, kind="ExternalInput").ap()
    onesrow_d = nc.dram_tensor("onesrow", (1, 128), BF16, kind="ExternalInput").ap()
    out_d = nc.dram_tensor("out", (128, BL * L), BF16, kind="ExternalOutput").ap()

    # ---- persistent SBUF ----------------------------------------------
    whaT = nc.alloc_sbuf_tensor("whaT_sb", [128, 512], BF16).ap()
    wa1r = nc.alloc_sbuf_tensor("wa1r_sb", [128, 256], BF16).ap()
    winitT = nc.alloc_sbuf_tensor("winitT_sb", [128, 512], BF16).ap()
    binit = nc.alloc_sbuf_tensor("binit_sb", [128, 2], F32).ap()
    wgsT = nc.alloc_sbuf_tensor("wgsT_sb", [128, 1536], BF16).ap()
    wgyT = nc.alloc_sbuf_tensor("wgyT_sb", [128, 768], BF16).ap()
    wgciT = nc.alloc_sbuf_tensor("wgciT_sb", [128, 1536], BF16).ap()
    wrsT = nc.alloc_sbuf_tensor("wrsT_sb", [128, 256], BF16).ap()
    wryT = nc.alloc_sbuf_tensor("wryT_sb", [128, 128], BF16).ap()
    wrciT = nc.alloc_sbuf_tensor("wrciT_sb", [128, 256], BF16).ap()
    biasgT = nc.alloc_sbuf_tensor("biasgT_sb", [128, 6], F32).ap()
    biasrT = nc.alloc_sbuf_tensor("biasrT_sb", [128, 1], F32).ap()
    ident = nc.alloc_sbuf_tensor("ident_sb", [128, 128], BF16).ap()
    onescol = nc.alloc_sbuf_tensor("onescol_sb", [128, 1], BF16).ap()
    onesrow = nc.alloc_sbuf_tensor("onesrow_sb", [1, 128], BF16).ap()

    h0T = nc.alloc_sbuf_tensor("h0T", [128, 16], BF16).ap()      # h[:,0,:] cols c*8+b
    partials = nc.alloc_sbuf_tensor("partials", [128, 8], F32).ap()
    ciT = nc.alloc_sbuf_tensor("ciT", [128, 16], BF16).ap()      # cols c*8+b
    s0T = nc.alloc_sbuf_tensor("s0T", [128, 16], BF16).ap()
    constgT = nc.alloc_sbuf_tensor("constgT", [128, 48], F32).ap()
    constrT = nc.alloc_sbuf_tensor("constrT", [128, 8], F32).ap()
    out_all = nc.alloc_sbuf_tensor("out_all", [128, BL * L], BF16).ap()

    with tile.TileContext(nc) as tc:
        # weight loads
        for sb, dr in [(whaT, whaT_d), (wa1r, wa1r_d), (winitT, winitT_d),
                       (binit, binit_d), (wgsT, wgsT_d), (wgyT, wgyT_d),
                       (wgciT, wgciT_d), (wrsT, wrsT_d), (wryT, wryT_d),
                       (wrciT, wrciT_d), (biasgT, biasgT_d), (biasrT, biasrT_d),
                       (ident, ident_d), (onescol, onescol_d),
                       (onesrow, onesrow_d)]:
            nc.sync.dma_start(sb[:, :], dr[:, :])

        # ================= Phase A =================
        with tc.tile_pool(name="pcit", bufs=1, space="PSUM") as pcit_pool:
          pciT0 = pcit_pool.tile([128, 8], F32, name="pciT0", tag="pciT0")
          pciT1 = pcit_pool.tile([128, 8], F32, name="pciT1", tag="pciT1")
          with tc.tile_pool(name="hnat", bufs=24) as hnat_pool, \
             tc.tile_pool(name="hts", bufs=6) as ht_pool, \
             tc.tile_pool(name="sba", bufs=3) as sba_pool, \
             tc.tile_pool(name="smalla", bufs=3) as sm_pool, \
             tc.tile_pool(name="ptr", bufs=2, space="PSUM") as ptr_pool, \
             tc.tile_pool(name="pwh", bufs=2, space="PSUM") as pwh_pool, \
             tc.tile_pool(name="pci", bufs=2, space="PSUM") as pci_pool:

            for b in range(BL):
                hn_tiles = []
                twh = sm_pool.tile([128, 16], F32, name=f"twh{b}", tag="twh")
                for i in range(NT):
                    hn = hnat_pool.tile([128, 256], BF16, name=f"hn{b}_{i}", tag="hn")
                    hn_tiles.append(hn)
                    nc.sync.dma_start(hn[:, :], h_d[b, bass.ts(i, 128), :])
                    # transpose both d-halves: (128t,128d) -> (128d,128t)
                    pt0 = ptr_pool.tile([128, 128], BF16, name=f"pt0_{b}_{i}", tag="pt")
                    pt1 = ptr_pool.tile([128, 128], BF16, name=f"pt1_{b}_{i}", tag="pt")
                    nc.tensor.transpose(pt0[:, :], hn[:, 0:128], ident[:, :])
                    nc.tensor.transpose(pt1[:, :], hn[:, 128:256], ident[:, :])
                    ht0 = ht_pool.tile([128, 128], BF16, name=f"ht0_{b}_{i}", tag="ht0")
                    ht1 = ht_pool.tile([128, 128], BF16, name=f"ht1_{b}_{i}", tag="ht1")
                    nc.vector.tensor_copy(ht0[:, :], pt0[:, :])
                    nc.scalar.copy(ht1[:, :], pt1[:, :])
                    if i == 0:
                        nc.vector.tensor_copy(h0T[:, b:b + 1], ht0[:, 0:1])
                        nc.vector.tensor_copy(h0T[:, 8 + b:8 + b + 1], ht1[:, 0:1])
                    # wh = h @ W_h_a.T for this chunk: (128t, 256h)
                    pw = pwh_pool.tile([128, 256], F32, name=f"pw{b}_{i}", tag="pw")
                    nc.tensor.matmul(pw[:, :], ht0[:, :], whaT[:, 0:256],
                                     start=True, stop=False)
                    nc.tensor.matmul(pw[:, :], ht1[:, :], whaT[:, 256:512],
                                     start=False, stop=True)
                    th = sba_pool.tile([128, 256], BF16, name=f"th{b}_{i}", tag="th")
                    nc.scalar.activation(th[:, :], pw[:, :], AF.Tanh)
                    tw = sba_pool.tile([128, 256], BF16, name=f"tw{b}_{i}", tag="tw")
                    nc.vector.tensor_mul(tw[:, :], th[:, :], wa1r[:, :])
                    nc.vector.reduce_sum(twh[:, i:i + 1], tw[:, :], axis=AX.X)

                # e = exp(twh) (unnormalized) via exp(x) = (1+t)/(1-t),
                # t = tanh(x/2): the tanh table is ~100x more accurate than
                # the exp table (act_info err 4 vs 400; native Exp measured
                # 3.9e-2 end-to-end rel err vs 2e-3 with the identity).
                tt = sm_pool.tile([128, 16], F32, name=f"tt{b}", tag="tt")
                nc.scalar.activation(tt[:, :], twh[:, :], AF.Tanh, scale=0.5)
                uu = sm_pool.tile([128, 16], F32, name=f"uu{b}", tag="uu")
                nc.vector.tensor_scalar_add(uu[:, :], tt[:, :], 1.0)
                ww = sm_pool.tile([128, 16], F32, name=f"ww{b}", tag="ww")
                nc.vector.tensor_scalar(ww[:, :], tt[:, :], -1.0, 1.0,
                                        OP.mult, OP.add)
                rw = sm_pool.tile([128, 16], F32, name=f"rw{b}", tag="rw")
                nc.vector.reciprocal(rw[:, :], ww[:, :])
                ee = sm_pool.tile([128, 16], BF16, name=f"ee{b}", tag="ee")
                nc.vector.tensor_mul(ee[:, :], uu[:, :], rw[:, :])
                nc.vector.reduce_sum(partials[:, b:b + 1], ee[:, :], axis=AX.X)
                # unnormalized ci: (1,256) psum accumulated over chunks
                pci = pci_pool.tile([1, 256], F32, name=f"pci{b}", tag="pci")
                for i in range(NT):
                    nc.tensor.matmul(pci[:, :], ee[:, i:i + 1], hn_tiles[i][:, :],
                                     start=(i == 0), stop=(i == NT - 1))
                # route the (1,256) ci row into columns of (128,8) psum tiles
                cis = sm_pool.tile([1, 256], BF16, name=f"cis{b}", tag="cis")
                nc.vector.tensor_copy(cis[:, :], pci[:, :])
                nc.tensor.matmul(pciT0[:, b:b + 1], cis[0:1, 0:128],
                                 onescol[0:1, 0:1], start=True, stop=True)
                nc.tensor.matmul(pciT1[:, b:b + 1], cis[0:1, 128:256],
                                 onescol[0:1, 0:1], start=True, stop=True)

          # ---- phase A wrap-up ----
          with tc.tile_pool(name="wrap", bufs=2) as wr_pool, \
               tc.tile_pool(name="pwr", bufs=1, space="PSUM") as pwr_pool:
              # S_b = sum over partitions of partials[:, b] via ones matmul
              pb16 = wr_pool.tile([128, 8], BF16, name="pb16", tag="pb16")
              nc.vector.tensor_copy(pb16[:, :], partials[:, :])
              psums = pwr_pool.tile([1, 8], F32, name="psums", tag="psums")
              nc.tensor.matmul(psums[:, :], onescol[:, :], pb16[:, :],
                               start=True, stop=True)
              rS = wr_pool.tile([1, 8], F32, name="rS", tag="rS")
              nc.vector.reciprocal(rS[:, :], psums[:, :])
              rSb = wr_pool.tile([1, 8], BF16, name="rSb", tag="rSb")
              nc.vector.tensor_copy(rSb[:, :], rS[:, :])
              prS = pwr_pool.tile([128, 8], F32, name="prS", tag="prS")
              nc.tensor.matmul(prS[:, :], onesrow[:, :], rSb[:, :],
                               start=True, stop=True)
              rSs = wr_pool.tile([128, 8], F32, name="rSs", tag="rSs")
              nc.vector.tensor_copy(rSs[:, :], prS[:, :])
              # normalize ci columns -> ciT (128, 16) bf16
              # (DVE tensor_tensor can read at most one PSUM operand)
              nc.vector.tensor_mul(ciT[:, 0:8], pciT0[:, :], rSs[:, :])
              nc.vector.tensor_mul(ciT[:, 8:16], pciT1[:, :], rSs[:, :])
              # s0T = tanh(W_init @ h0 + b_init) in T layout
              for j in range(2):
                  ps0 = pwr_pool.tile([128, 8], F32, name=f"ps0{j}", tag="ps0")
                  nc.tensor.matmul(ps0[:, :], winitT[:, j * 128:j * 128 + 128],
                                   h0T[:, 0:8], start=True, stop=False)
                  nc.tensor.matmul(ps0[:, :], winitT[:, 256 + j * 128:256 + j * 128 + 128],
                                   h0T[:, 8:16], start=False, stop=True)
                  nc.scalar.activation(s0T[:, bass.ts(j, 8)], ps0[:, :], AF.Tanh,
                                       bias=binit[:, j:j + 1])
              # constgT = ci @ Wg_ci.T + biases (T layout, 6 f-tiles)
              for jj in range(6):
                  pcg = pwr_pool.tile([128, 8], F32, name=f"pcg{jj}", tag="pcg")
                  nc.tensor.matmul(pcg[:, :], wgciT[:, jj * 128:jj * 128 + 128],
                                   ciT[:, 0:8], start=True, stop=False)
                  nc.tensor.matmul(pcg[:, :], wgciT[:, 768 + jj * 128:768 + jj * 128 + 128],
                                   ciT[:, 8:16], start=False, stop=True)
                  nc.scalar.activation(constgT[:, bass.ts(jj, 8)], pcg[:, :],
                                       AF.Identity, bias=biasgT[:, jj:jj + 1])
              pcr = pwr_pool.tile([128, 8], F32, name="pcr", tag="pcr")
              nc.tensor.matmul(pcr[:, :], wrciT[:, 0:128], ciT[:, 0:8],
                               start=True, stop=False)
              nc.tensor.matmul(pcr[:, :], wrciT[:, 128:256], ciT[:, 8:16],
                               start=False, stop=True)
              nc.scalar.activation(constrT[:, :], pcr[:, :], AF.Identity,
                                   bias=biasrT[:, :])

        # ================= Phase B =================
        # PSUM accumulation-group discipline: start_tensor_calc=True wipes
        # the has-written state for the WHOLE bank, so a region must never
        # be accumulated into after another group opened in its bank.
        # Every region below is written by one CONSECUTIVE group and only
        # read afterwards.  Layout within the per-step (128,128) f32 bank:
        #   [0:32)   przs  rz gates, s contribution
        #   [32:48)  pnis  gh_n (s contribution)
        #   [48:80)  pyrz  rz gates, y contribution
        #   [80:96)  pyn   i_n y contribution
        #   [96:104) prn   RNN pre-activation
        #   [104:112) pbb  1/S broadcast   row0 [112:120) ps: sum(ey)
        outv = out_all.rearrange("p (b t) -> p b t", t=L)
        with tc.tile_pool(name="pb", bufs=4, space="PSUM") as pb_pool, \
             tc.tile_pool(name="sbb", bufs=4) as sbb_pool, \
             tc.tile_pool(name="stp", bufs=3) as st_pool:

            sT = s0T
            yT = None
            pend = None  # (ey tile, psum tile, t) awaiting softmax tail

            for t in range(L):
                pbt = pb_pool.tile([128, 128], F32, name=f"pbt{t}", tag="pb")
                # --- gate matmuls, s contributions (closed groups) ---
                for jj in range(4):
                    sl = pbt[:, jj * 8:jj * 8 + 8]
                    nc.tensor.matmul(sl, wgsT[:, jj * 128:jj * 128 + 128],
                                     sT[:, 0:8], start=True, stop=False)
                    nc.tensor.matmul(sl, wgsT[:, 768 + jj * 128:768 + jj * 128 + 128],
                                     sT[:, 8:16], start=False, stop=True)
                for jj in range(2):
                    sl = pbt[:, 32 + jj * 8:32 + jj * 8 + 8]
                    nc.tensor.matmul(sl, wgsT[:, (4 + jj) * 128:(4 + jj) * 128 + 128],
                                     sT[:, 0:8], start=True, stop=False)
                    nc.tensor.matmul(sl, wgsT[:, 768 + (4 + jj) * 128:768 + (4 + jj) * 128 + 128],
                                     sT[:, 8:16], start=False, stop=True)

                # --- previous step's softmax tail ---
                rzin = sbb_pool.tile([128, 32], F32, name=f"rzin{t}", tag="rzin")
                if pend is not None:
                    ey_p, pbt_p, t_p = pend
                    ps = pbt_p[0:1, 112:120]
                    nc.tensor.matmul(ps, onescol[:, :], ey_p[:, :],
                                     start=True, stop=True)
                    rs = sbb_pool.tile([1, 8], F32, name=f"rs{t}", tag="rs")
                    nc.vector.reciprocal(rs[:, :], ps)
                    rsb = sbb_pool.tile([1, 8], BF16, name=f"rsb{t}", tag="rsb")
                    nc.vector.tensor_copy(rsb[:, :], rs[:, :])
                    pbb = pbt_p[:, 104:112]
                    nc.tensor.matmul(pbb, onesrow[:, :], rsb[:, :],
                                     start=True, stop=True)
                    # rzin only needs przs -- emit before yout to overlap
                    nc.vector.tensor_add(rzin[:, :], pbt[:, 0:32],
                                         constgT[:, 0:32])
                    nc.vector.tensor_mul(outv[:, :, t_p], ey_p[:, :], pbb)
                    yT = outv[:, :, t_p]
                    pend = None
                else:
                    nc.vector.tensor_add(rzin[:, :], pbt[:, 0:32],
                                         constgT[:, 0:32])

                # --- gate matmuls, y contributions (own closed groups) ---
                if yT is not None:
                    for jj in range(4):
                        nc.tensor.matmul(pbt[:, 48 + jj * 8:48 + jj * 8 + 8],
                                         wgyT[:, jj * 128:jj * 128 + 128],
                                         yT, start=True, stop=True)
                    for jj in range(2):
                        nc.tensor.matmul(pbt[:, 80 + jj * 8:80 + jj * 8 + 8],
                                         wgyT[:, (4 + jj) * 128:(4 + jj) * 128 + 128],
                                         yT, start=True, stop=True)

                # --- GRU elementwise (T layout) ---
                if yT is not None:
                    rzin2 = sbb_pool.tile([128, 32], F32, name=f"rzin2{t}", tag="rzin2")
                    nc.vector.tensor_add(rzin2[:, :], rzin[:, :], pbt[:, 48:80])
                else:
                    rzin2 = rzin
                # sigmoid emulated as 0.5*tanh(0.5x)+0.5 (keeps everything in
                # act table set 0, which also holds exp for the softmax).
                # r first: it gates the n-path; z's ops fill engine bubbles.
                trz = sbb_pool.tile([128, 32], F32, name=f"trz{t}", tag="trz")
                nc.scalar.activation(trz[:, 0:16], rzin2[:, 0:16], AF.Tanh,
                                     scale=0.5)
                nc.scalar.activation(trz[:, 16:32], rzin2[:, 16:32], AF.Tanh,
                                     scale=0.5)
                sig = sbb_pool.tile([128, 32], F32, name=f"sig{t}", tag="sig")
                nc.vector.tensor_scalar(sig[:, 0:16], trz[:, 0:16], 0.5, 0.5,
                                        OP.mult, OP.add)
                ns = sbb_pool.tile([128, 16], F32, name=f"ns{t}", tag="ns")
                if yT is None:
                    rh = sbb_pool.tile([128, 16], F32, name=f"rh{t}", tag="rh")
                    nc.vector.tensor_mul(rh[:, :], sig[:, 0:16], pbt[:, 32:48])
                    nc.vector.tensor_add(ns[:, :], rh[:, :], constgT[:, 32:48])
                else:
                    nin = sbb_pool.tile([128, 16], F32, name=f"nin{t}", tag="nin")
                    nc.vector.tensor_add(nin[:, :], pbt[:, 80:96], constgT[:, 32:48])
                    rh = sbb_pool.tile([128, 16], F32, name=f"rh{t}", tag="rh")
                    nc.vector.tensor_mul(rh[:, :], sig[:, 0:16], pbt[:, 32:48])
                    nc.vector.tensor_add(ns[:, :], rh[:, :], nin[:, :])
                nc.vector.tensor_scalar(sig[:, 16:32], trz[:, 16:32], 0.5, 0.5,
                                        OP.mult, OP.add)
                nn = sbb_pool.tile([128, 16], BF16, name=f"nn{t}", tag="nn")
                nc.scalar.activation(nn[:, :], ns[:, :], AF.Tanh)
                d1 = sbb_pool.tile([128, 16], BF16, name=f"d1_{t}", tag="d1")
                nc.vector.tensor_sub(d1[:, :], sT[:, :], nn[:, :])
                d2 = sbb_pool.tile([128, 16], BF16, name=f"d2_{t}", tag="d2")
                nc.vector.tensor_mul(d2[:, :], sig[:, 16:32], d1[:, :])
                sTn = st_pool.tile([128, 16], BF16, name=f"sT{t}", tag="sT")
                nc.vector.tensor_add(sTn[:, :], nn[:, :], d2[:, :])

                # --- RNN cell + exp (one consecutive group) ---
                prn = pbt[:, 96:104]
                nc.tensor.matmul(prn, wrsT[:, 0:128], sTn[:, 0:8],
                                 start=True, stop=False)
                nc.tensor.matmul(prn, wrsT[:, 128:256], sTn[:, 8:16],
                                 start=False, stop=(yT is None))
                if yT is not None:
                    nc.tensor.matmul(prn, wryT[:, :], yT,
                                     start=False, stop=True)
                rin = sbb_pool.tile([128, 8], F32, name=f"rin{t}", tag="rin")
                nc.vector.tensor_add(rin[:, :], prn, constrT[:, :])
                vv = sbb_pool.tile([128, 8], F32, name=f"vv{t}", tag="vv")
                nc.scalar.activation(vv[:, :], rin[:, :], AF.Tanh)
                # native exp: vv = tanh(..) is in [-1,1], where the exp table
                # is accurate enough (the wide-domain phase A softmax keeps
                # the tanh-identity form)
                ey = sbb_pool.tile([128, 8], BF16, name=f"ey{t}", tag="ey")
                nc.scalar.activation(ey[:, :], vv[:, :], AF.Exp)

                pend = (ey, pbt, t)
                sT = sTn

            # final step's softmax tail
            ey_p, pbt_p, t_p = pend
            ps = pbt_p[0:1, 112:120]
            nc.tensor.matmul(ps, onescol[:, :], ey_p[:, :], start=True, stop=True)
            rs = sbb_pool.tile([1, 8], F32, name="rsF", tag="rs")
            nc.vector.reciprocal(rs[:, :], ps)
            rsb = sbb_pool.tile([1, 8], BF16, name="rsbF", tag="rsb")
            nc.vector.tensor_copy(rsb[:, :], rs[:, :])
            pbb = pbt_p[:, 104:112]
            nc.tensor.matmul(pbb, onesrow[:, :], rsb[:, :], start=True, stop=True)
            nc.vector.tensor_mul(outv[:, :, t_p], ey_p[:, :], pbb)

        nc.sync.dma_start(out_d[:, :], out_all[:, :])

    nc.compile()
    return nc


def _pack_weights(inputs):
    f = np.float32
    bf = ml_dtypes.bfloat16
    W_h_a = np.asarray(inputs["W_h_a"], f)
    W_a = np.asarray(inputs["W_a"], f)
    W_init = np.asarray(inputs["W_init"], f)
    b_init = np.asarray(inputs["b_init"], f)
    W_ih_g = np.asarray(inputs["W_ih_g"], f)
    W_hh_g = np.asarray(inputs["W_hh_g"], f)
    b_ih_g = np.asarray(inputs["b_ih_g"], f)
    b_hh_g = np.asarray(inputs["b_hh_g"], f)
    W_ih_r = np.asarray(inputs["W_ih_r"], f)
    W_hh_r = np.asarray(inputs["W_hh_r"], f)
    b_ih_r = np.asarray(inputs["b_ih_r"], f)
    b_hh_r = np.asarray(inputs["b_hh_r"], f)

    assert not np.any(b_hh_g[512:]), "nonzero b_hh_g n-part not supported"

    def split2(m):  # (256, X) -> (128, 2X), k-chunks side by side
        return np.concatenate([m[0:128], m[128:256]], axis=1)

    wk = {}
    wk["whaT"] = split2(W_h_a.T).astype(bf)
    wk["wa1r"] = np.tile(W_a[0, :256][None, :], (128, 1)).astype(bf)
    wk["winitT"] = split2(W_init.T).astype(bf)
    wk["binit"] = np.ascontiguousarray(b_init.reshape(2, 128).T)
    wk["wgsT"] = split2(W_hh_g.T).astype(bf)
    wk["wgyT"] = W_ih_g[:, 256:384].T.astype(bf)
    wk["wgciT"] = split2(W_ih_g[:, 0:256].T).astype(bf)
    wk["wrsT"] = split2(W_ih_r[:, 256:512].T).astype(bf)
    wk["wryT"] = W_hh_r.T.astype(bf)
    wk["wrciT"] = split2(W_ih_r[:, 0:256].T).astype(bf)
    bias_g = b_ih_g + np.concatenate([b_hh_g[:512], np.zeros(256, f)])
    wk["biasgT"] = np.ascontiguousarray(bias_g.reshape(6, 128).T)
    wk["biasrT"] = np.ascontiguousarray((b_ih_r + b_hh_r).reshape(128, 1))
    wk["ident"] = np.eye(128, dtype=bf)
    wk["onescol"] = np.ones((128, 1), bf)
    wk["onesrow"] = np.ones((1, 128), bf)
    return {k: np.ascontiguousarray(v) for k, v in wk.items()}


def run(inputs, trace=False):
    from concourse import bass_utils

    assert int(inputs["out_len"]) == L
    if "nc" not in _CACHE:
        _CACHE["nc"] = _build_program()
    nc = _CACHE["nc"]

    wk = _pack_weights(inputs)
    h = np.asarray(inputs["h"], np.float32).astype(ml_dtypes.bfloat16)
    in_maps = []
    for c in range(NC):
        m = dict(wk)
        m["h"] = np.ascontiguousarray(h[c * BL:(c + 1) * BL])
        in_maps.append(m)

    res = bass_utils.run_bass_kernel_spmd(
        nc, in_maps, core_ids=list(range(NC)), trace=trace)

    out = np.empty((B, L, DO), np.float32)
    for c in range(NC):
        r = np.asarray(res.results[c]["out"]).astype(np.float32)
        r = r.reshape(128, BL, L)
        out[c * BL:(c + 1) * BL] = r.transpose(1, 2, 0)
    return out, res


def kernel(**inputs):
    out, _ = run(inputs, trace=False)
    return out
